# revision 31
# baseline (speedup 1.0000x reference)
"""Trainium2 Bass kernel for nn_Attention_3135326126702.

Computation (see reference): ViT-style attention block on x:(2,384,56,56).
  q/k/v/proj are 1x1 conv + eval-mode BN (affine, folded into weights on host).
  8 heads, key_dim=16, d=64, N=3136 positions, softmax(q@k) (no scale) @ v,
  relu, proj conv+BN.

Sharding: 8 cores = (batch b in {0,1}) x (head-pair hp in {0..3}).
Each core computes K/V/Q convs for ITS 2 HEADS only over all N positions
(no duplicated conv work), runs attention for its 2 heads over 4 query
chunks of 784, and projects its 2 heads' contribution to the output --
a PARTIAL [384, N] result. The host sums the 4 partials per batch and
adds the proj bias: proj is linear in heads, so no device collective.

On-chip layout (channels/keys on partitions, no transposes):
  The head block-diagonal lives on the Q side: q_sb[:, h, :] has head h's
  16 q-channels at rows 16*(3h) (h0 rows 0:16, h1 rows 48:64), zeros
  elsewhere; k_all[128, N] holds both heads' K on the matching rows.
  Every scores matmul is a full K=128 contraction (keeps the PE HAM
  clock-gate at 8/8); zero rows make it per-head.
    scoresT[key, q] = matmul(lhsT=k_all[:, key_tile], rhs=q_sb[:, h, qchunk])
  exp is split across TWO engines (the loop is otherwise ScalarE-gated):
  16/25 key tiles use ScalarE exp; 9/25 use a DVE Schraudolph
  approximation -- one tensor_scalar op computing
  int16(s*184.665 + 16251.05), whose bit pattern IS bf16 exp(s) to ~3%
  (softmax ratio structure cancels most of it; measured end-to-end
  rel err ~7e-3 vs the 2e-2 gate).
  xxT_aug[65, q] += matmul(lhsT=VT_aug[key_tile, h, 65] bf16, rhs=exp_scoresT)
    VT_aug col 64 == 1.0 -> row 64 accumulates the softmax denominator.
  normalize: denominator row -> DRAM-bounce partition broadcast,
  fast-approx reciprocal, xx = max(xxT*rec + bv, 0).
  proj partial per query chunk (2 K=64 matmuls/tile), streamed out by DMA
  during the next chunk's attention; only the last chunk is tail.

dtypes: convs/scores/proj fp16, attnV bf16 (exp values reach ~1e7),
partial outputs f32 (summed on host).
"""

import numpy as np

import concourse.bass as bass
import concourse.mybir as mybir
from concourse import bacc
import concourse.tile as tile
from concourse.bass_utils import run_bass_kernel_spmd

F32 = mybir.dt.float32
BF16 = mybir.dt.bfloat16
FP16 = mybir.dt.float16
I16 = mybir.dt.int16
AF = mybir.ActivationFunctionType
ALU = mybir.AluOpType

EPS = 1e-5
B = 2
CIN = 384          # input channels
N = 3136           # 56*56 positions
NQ = 784           # query positions per chunk (N/4)
NH = 8             # heads total
NHL = 2            # heads per core
KD = 16            # key dim per head
NKT = 25           # key tiles of 128 (last has 64)
FCH = ((0, 512), (512, 272))  # 784-wide free dim split at the PSUM bank edge
N_CORES = 8
ROW = (0, 64)      # q/k channel row offset per local head (32-aligned)

# Schraudolph bf16 exp: bits = trunc(s * 2^7/ln2 + (127*2^7 - 5.45 + 0.5))
EXP_A = 184.6649652337873
EXP_B = 16251.05
# exp engine alternates per key tile: odd -> DVE, even -> ScalarE. With the
# lag-2 attnV emission this keeps both exp engines concurrently busy while
# PE streams matmuls back-to-back.
DVE_KT = frozenset(range(1, 24, 2))

_CACHE = {}


def _key_tiles():
    for kt in range(NKT):
        p0 = kt * 128
        yield kt, p0, min(128, N - p0)


def build_nc() -> bass.Bass:
    nc = bacc.Bacc()

    xb = nc.declare_dram_parameter("xb", [CIN, N], FP16, isOutput=False)[:]
    wqT = nc.declare_dram_parameter("wqT", [CIN, 128], FP16, isOutput=False)[:]
    wkT = nc.declare_dram_parameter("wkT", [CIN, 128], FP16, isOutput=False)[:]
    wvT = nc.declare_dram_parameter("wvT", [CIN, 128], FP16, isOutput=False)[:]
    wpT = nc.declare_dram_parameter("wpT", [64, NHL, CIN], FP16, isOutput=False)[:]
    bq = nc.declare_dram_parameter("bq", [128, 1], F32, isOutput=False)[:]
    bv2 = nc.declare_dram_parameter("bv2", [64, NHL], F32, isOutput=False)[:]
    out = nc.declare_dram_parameter("out", [CIN, N], F32, isOutput=True)[:]

    with tile.TileContext(nc) as tc:
        with (
            tc.tile_pool(name="const", bufs=1) as cst,
            tc.tile_pool(name="work", bufs=3) as wk,
            tc.tile_pool(name="ps", bufs=2, space="PSUM") as ps,
            tc.tile_pool(name="dram", bufs=2, space="DRAM") as dr,
        ):
            # ---- constants / weights ----
            # DMA priority: k/q weights and the first x column group come
            # first so the convs (and with them the PE clock ramp) start
            # as early as possible; everything else follows.
            wk_sb = cst.tile([128, 3, 128], FP16)
            nc.sync.dma_start(out=wk_sb, in_=wkT.rearrange("(o p) m -> p o m", p=128))

            # ---- persistent activations ----
            # q_sb[:, h]: head h q-channels at rows ROW[h]..+16, zeros elsewhere
            q_sb = cst.tile([128, NHL, N], FP16)
            k_all = cst.tile([128, N], FP16)       # both heads' K on ROW rows
            vvT_sb = cst.tile([128, NKT, NHL, 65], BF16)  # [key, kt, h, 64 V + ones]
            xx_sb = cst.tile([64, NHL, NQ], FP16)  # per-chunk normalized xx

            nc.vector.memset(k_all.bitcast(F32), 0.0)
            nc.vector.memset(q_sb[:, :, 0:NQ].bitcast(F32), 0.0)
            nc.vector.memset(vvT_sb[:, :, :, 64:65], 1.0)
            for qd in range(1, 4):
                nc.vector.memset(q_sb[:, :, qd * NQ:(qd + 1) * NQ].bitcast(F32), 0.0)

            x_sb = cst.tile([128, 3, N], FP16)
            x_src = xb.rearrange("(o p) n -> p o n", p=128)
            nc.sync.dma_start(out=x_sb[:, 0, 0:NQ], in_=x_src[:, 0, 0:NQ])
            nc.sync.dma_start(out=x_sb[:, 1:3, 0:NQ], in_=x_src[:, 1:3, 0:NQ])

            wq_sb = cst.tile([128, 3, 128], FP16)
            nc.sync.dma_start(out=wq_sb, in_=wqT.rearrange("(o p) m -> p o m", p=128))
            bq_sb = cst.tile([128, 1], F32)
            nc.sync.dma_start(out=bq_sb, in_=bq)
            wv_sb = cst.tile([128, 3, 128], FP16)
            nc.sync.dma_start(out=wv_sb, in_=wvT.rearrange("(o p) m -> p o m", p=128))
            wp_sb = cst.tile([64, NHL, CIN], FP16)
            nc.sync.dma_start(out=wp_sb, in_=wpT)
            bv2_sb = cst.tile([64, NHL], F32)
            nc.sync.dma_start(out=bv2_sb, in_=bv2)
            for qd in range(1, 4):
                nc.sync.dma_start(
                    out=x_sb[:, :, qd * NQ:(qd + 1) * NQ],
                    in_=x_src[:, :, qd * NQ:(qd + 1) * NQ],
                )

            # ---- emission order == scheduler priority ----

            def k_conv(g):
                # K conv for column group g -> one evacuation copy into k_all
                # (BN bias dropped: per-query-constant shift is softmax-invariant)
                g0 = g * NQ
                for f0, fl in FCH:
                    kp = ps.tile([128, NQ], F32, tag="sc", bufs=3, name="kp")
                    for c in range(3):
                        nc.tensor.matmul(
                            kp[:, :fl],
                            lhsT=wk_sb[:, c, :],
                            rhs=x_sb[:, c, g0 + f0:g0 + f0 + fl],
                            start=(c == 0), stop=(c == 2),
                        )
                    nc.vector.tensor_copy(
                        out=k_all[:, g0 + f0:g0 + f0 + fl], in_=kp[:, :fl],
                    )

            def q_conv(qc):
                # Q conv for one query chunk; scatter the 2 head stripes
                # (same partitions -- engines cannot shift partitions).
                g0 = qc * NQ
                for f0, fl in FCH:
                    qp = ps.tile([128, NQ], F32, tag="sc", bufs=3, name="qp")
                    for c in range(3):
                        nc.tensor.matmul(
                            qp[:, :fl],
                            lhsT=wq_sb[:, c, :],
                            rhs=x_sb[:, c, g0 + f0:g0 + f0 + fl],
                            start=(c == 0), stop=(c == 2),
                        )
                    for h in range(NHL):
                        r = ROW[h]
                        nc.vector.tensor_scalar(
                            out=q_sb[r:r + KD, h, g0 + f0:g0 + f0 + fl],
                            in0=qp[r:r + KD, :fl],
                            scalar1=bq_sb[r:r + KD, 0:1], scalar2=None,
                            op0=ALU.add,
                        )

            def vt_conv(kt, p0, pl):
                # VT conv: out[key_tile, 128] = x_chunk.T @ wv.T. BN bias
                # added after normalization (softmax weights sum to 1).
                vp = ps.tile([128, 128], F32, tag="sc", bufs=3, name="vp")
                for c in range(3):
                    nc.tensor.matmul(
                        vp[:pl, :],
                        lhsT=x_sb[:, c, p0:p0 + pl],
                        rhs=wv_sb[:, c, :],
                        start=(c == 0), stop=(c == 2),
                    )
                nc.vector.tensor_copy(
                    out=vvT_sb[:pl, kt, :, 0:64],
                    in_=vp[:pl, :].rearrange("p (h d) -> p h d", h=NHL),
                )

            # ONE software-pipelined PE stream across ALL units: attnV for
            # slot s is emitted interleaved with the scores of slot s+2
            # (S512(k), A512(k-2), S272(k), A272(k-2)) so that (a) PE never
            # sits out an exp -- it has had ~2 slots of wall clock -- and
            # (b) every 151ns scores-LDWEIGHTS hides under a 512-wide
            # matmul. The lag queue carries across unit boundaries, so unit
            # transitions cost nothing.
            lag = []

            def attn_chunk(xx2, h, es, kt, pl, f0, fl):
                nc.tensor.matmul(
                    xx2[0] if f0 == 0 else xx2[1],
                    lhsT=vvT_sb[:pl, kt, h, :],
                    rhs=es[:pl, f0:f0 + fl],
                    start=(kt == 0), stop=(kt == NKT - 1),
                )

            def unit(h, qc, side_work):
                # the two attnV accumulators are single-buffered one-bank
                # tiles (evacuated to SBUF right after their last attnV by
                # ScalarE), which frees PSUM for sp bufs=3: the chain
                # scores(k) -> exp(k) -> sem -> scores(k+3) then has
                # positive slack and PE streams at full rate.
                q0 = qc * NQ
                xxA = ps.tile([65, 512], F32, tag="xa", bufs=1, name="xxA")
                xxB = ps.tile([65, 272], F32, tag="xb", bufs=1, name="xxB")
                for kt, p0, pl in _key_tiles():
                    if kt in side_work:
                        side_work[kt]()
                    sp = ps.tile([128, NQ], F32, tag="sc", bufs=3, name="sp")
                    nc.tensor.matmul(
                        sp[:pl, 0:512],
                        lhsT=k_all[:, p0:p0 + pl],
                        rhs=q_sb[:, h, q0:q0 + 512],
                        start=True, stop=True,
                    )
                    if len(lag) > 2:
                        attn_chunk(*lag[0], 0, 512)
                    nc.tensor.matmul(
                        sp[:pl, 512:784],
                        lhsT=k_all[:, p0:p0 + pl],
                        rhs=q_sb[:, h, q0 + 512:q0 + NQ],
                        start=True, stop=True,
                    )
                    es = wk.tile([128, NQ], BF16, tag="es", bufs=8, name="es")
                    if kt in DVE_KT:
                        with nc.allow_low_precision(reason="schraudolph exp ~3%"):
                            nc.vector.tensor_scalar(
                                out=es[:pl].bitcast(I16), in0=sp[:pl],
                                scalar1=EXP_A, scalar2=EXP_B,
                                op0=ALU.mult, op1=ALU.add,
                            )
                    else:
                        nc.scalar.activation(out=es[:pl], in_=sp[:pl], func=AF.Exp)
                    if len(lag) > 2:
                        attn_chunk(*lag.pop(0), 512, 272)
                    lag.append(((xxA, xxB), h, es, kt, pl))
                return (xxA, xxB)

            def flush_lag():
                for args in lag:
                    for f0, fl in FCH:
                        attn_chunk(*args, f0, fl)
                lag.clear()

            def evac_parts(xx2):
                # ScalarE copies of the finished accumulators into SBUF;
                # frees the single-buffered PSUM tiles for the next unit
                xxs = wk.tile([65, NQ], F32, tag="xxs", bufs=2, name="xxs")

                def eA():
                    nc.scalar.copy(out=xxs[:, 0:512], in_=xx2[0])

                def eB():
                    nc.scalar.copy(out=xxs[:, 512:784], in_=xx2[1])

                return xxs, eA, eB

            def norm_parts(xxs, h):
                # normalize + relu: xx = max(xxT*rec + bv, 0); denom = row 64.
                # Returned as FOUR side-work pieces spread across the next
                # unit's key tiles, so each ~1us DVE op slots between that
                # unit's exps instead of pushing them past their attnV
                # deadline. The DRAM-bounce broadcast latency is covered by
                # the gap between piece 1 and piece 2.
                st = {}

                def p1():
                    dnd = dr.tile([1, NQ], F32, name="dnd")
                    nc.sync.dma_start(out=dnd, in_=xxs[64:65, :])
                    st["dnb"] = wk.tile([64, NQ], F32, tag="dnb", bufs=2, name="dnb")
                    nc.gpsimd.dma_start(
                        out=st["dnb"], in_=dnd[0, :].partition_broadcast(64)
                    )

                def p2():
                    st["rec"] = wk.tile([64, NQ], F32, tag="rec", bufs=2, name="rec")
                    with nc.allow_low_precision(reason="~18-bit softmax recip"):
                        nc.vector.reciprocal_approx_fast(out=st["rec"], in_=st["dnb"])

                def p3():
                    st["xt"] = wk.tile([64, NQ], FP16, tag="xt", bufs=2, name="xt")
                    nc.vector.tensor_tensor(
                        out=st["xt"], in0=xxs[0:64, :], in1=st["rec"], op=ALU.mult,
                    )

                def p4():
                    nc.vector.tensor_scalar(
                        out=xx_sb[:, h, :], in0=st["xt"],
                        scalar1=bv2_sb[:, h:h + 1],
                        scalar2=0.0, op0=ALU.add, op1=ALU.max,
                    )

                return p1, p2, p3, p4

            def norm_tail(xxs, h):
                dnd = dr.tile([1, NQ], F32, name="dnd")
                nc.sync.dma_start(out=dnd, in_=xxs[64:65, :])
                dnb = wk.tile([64, NQ], F32, tag="dnb", bufs=2, name="dnb")
                nc.sync.dma_start(
                    out=dnb, in_=dnd[0, :].partition_broadcast(64)
                )
                rec = wk.tile([64, NQ], F32, tag="rec", bufs=2, name="rec")
                with nc.allow_low_precision(reason="~18-bit softmax recip"):
                    nc.vector.reciprocal_approx_fast(out=rec, in_=dnb)
                xt = wk.tile([64, NQ], FP16, tag="xt", bufs=2, name="xt")
                nc.vector.tensor_tensor(
                    out=xt, in0=xxs[0:64, :], in1=rec, op=ALU.mult,
                )
                nc.vector.tensor_scalar(
                    out=xx_sb[:, h, :], in0=xt, scalar1=bv2_sb[:, h:h + 1],
                    scalar2=0.0, op0=ALU.add, op1=ALU.max,
                )

            def proj_tile(qc, t):
                # partial proj (this core's 2 heads only) for one output tile
                pp = ps.tile([128, NQ], F32, tag="sc", bufs=3, name="pp")
                for f0, fl in FCH:
                    for h in range(NHL):
                        nc.tensor.matmul(
                            pp[:, f0:f0 + fl],
                            lhsT=wp_sb[:, h, 128 * t:128 * t + 128],
                            rhs=xx_sb[:, h, f0:f0 + fl],
                            start=(h == 0), stop=(h == NHL - 1),
                        )
                ob = wk.tile([128, NQ], F32, tag="ob", bufs=3, name="ob")
                nc.scalar.copy(out=ob, in_=pp)
                nc.sync.dma_start(
                    out=out.rearrange("(o p) n -> p o n", p=128)[
                        :, t, qc * NQ:(qc + 1) * NQ],
                    in_=ob,
                )

            k_conv(0)
            q_conv(0)

            def mknorm(xx2, h, extra=None):
                xxs, eA, eB = evac_parts(xx2)
                p1, p2, p3, p4 = norm_parts(xxs, h)
                sw = {3: eA, 4: eB, 6: p1, 10: p2, 12: p3, 14: p4}
                if extra:
                    sw.update(extra)
                return sw

            def mkproj(qc):
                return {16: lambda: proj_tile(qc, 0),
                        19: lambda: proj_tile(qc, 1),
                        22: lambda: proj_tile(qc, 2)}

            # first unit races the VT conv tile-by-tile and the remaining
            # K column groups (scores only reach group g's keys from kt~7g);
            # each unit's normalize runs as split side work in the NEXT unit
            sw0 = {kt: (lambda kt=kt, p0=p0, pl=pl: vt_conv(kt, p0, pl))
                   for kt, p0, pl in _key_tiles()}
            for g, kt in ((1, 2), (2, 8), (3, 14)):
                prev = sw0[kt]
                sw0[kt] = (lambda g=g, prev=prev: (prev(), k_conv(g)))
            xp = unit(0, 0, sw0)
            xp = unit(1, 0, mknorm(xp, 0, {18: lambda: q_conv(1)}))
            xp = unit(0, 1, mknorm(xp, 1, mkproj(0)))
            xp = unit(1, 1, mknorm(xp, 0, {18: lambda: q_conv(2)}))
            xp = unit(0, 2, mknorm(xp, 1, mkproj(1)))
            xp = unit(1, 2, mknorm(xp, 0, {18: lambda: q_conv(3)}))
            xp = unit(0, 3, mknorm(xp, 1, mkproj(2)))
            xp = unit(1, 3, mknorm(xp, 0))
            flush_lag()
            xxs_t, eA_t, eB_t = evac_parts(xp)
            eA_t()
            eB_t()
            norm_tail(xxs_t, 1)
            for t in range(3):
                proj_tile(3, t)

    nc.compile()
    return nc


def _fold_bn(w, g, b, m, v):
    s = (g / np.sqrt(v + EPS)).astype(np.float32)
    return (s[:, None] * w).astype(np.float32), (b - m * s).astype(np.float32)


def make_in_maps(inputs):
    """Host-side prep: fold BN, slice per head-pair, pad q/k rows."""
    wq, bq_ = _fold_bn(inputs["wq"], inputs["qg"], inputs["qb"], inputs["qm"], inputs["qv"])
    wkm, _ = _fold_bn(inputs["wk"], inputs["kg"], inputs["kb"], inputs["km"], inputs["kvv"])
    wv, bv_ = _fold_bn(inputs["wv"], inputs["vg"], inputs["vb"], inputs["vm"], inputs["vvv"])
    wp, _ = _fold_bn(inputs["wp"], inputs["pg"], inputs["pb"], inputs["pm"], inputs["pvv"])

    x = np.asarray(inputs["x"], np.float32).reshape(B, CIN, N)
    xb16 = [np.ascontiguousarray(x[b]).astype(np.float16) for b in range(B)]

    in_maps = []
    for core in range(N_CORES):
        b, hp = divmod(core, 4)
        h0 = 2 * hp
        # q/k weights for local heads at rows ROW[h]..+16 of a 128-row block
        wq2 = np.zeros((128, CIN), np.float32)
        wk2 = np.zeros((128, CIN), np.float32)
        bq2 = np.zeros((128, 1), np.float32)
        for hl in range(NHL):
            h = h0 + hl
            r = ROW[hl]
            wq2[r:r + KD] = wq[KD * h:KD * h + KD]
            wk2[r:r + KD] = wkm[KD * h:KD * h + KD]
            bq2[r:r + KD, 0] = bq_[KD * h:KD * h + KD]
        wv2 = wv[64 * h0:64 * h0 + 128]              # [128, CIN]
        wp2 = wp[:, 64 * h0:64 * h0 + 128]           # [CIN, 128]
        bv22 = bv_[64 * h0:64 * h0 + 128]
        in_maps.append({
            "xb": xb16[b],
            "wqT": np.ascontiguousarray(wq2.T).astype(np.float16),
            "wkT": np.ascontiguousarray(wk2.T).astype(np.float16),
            "wvT": np.ascontiguousarray(wv2.T).astype(np.float16),
            "wpT": np.ascontiguousarray(
                wp2.T.reshape(NHL, 64, CIN).transpose(1, 0, 2)).astype(np.float16),
            "bq": bq2,
            "bv2": np.ascontiguousarray(bv22.reshape(NHL, 64).T),  # [64, 2]
        })
    return in_maps


def assemble(results, inputs):
    _, bp_ = _fold_bn(inputs["wp"], inputs["pg"], inputs["pb"], inputs["pm"], inputs["pvv"])
    out = np.empty((B, CIN, N), np.float32)
    for b in range(B):
        acc = results[4 * b]["out"].astype(np.float32)
        for hp in range(1, 4):
            acc = acc + results[4 * b + hp]["out"]
        out[b] = acc + bp_[:, None]
    return out.reshape(B, CIN, 56, 56)


def kernel(**inputs) -> np.ndarray:
    if "nc" not in _CACHE:
        _CACHE["nc"] = build_nc()
    nc = _CACHE["nc"]
    in_maps = make_in_maps(inputs)
    res = run_bass_kernel_spmd(nc, in_maps, core_ids=list(range(N_CORES)))
    return assemble(res.results, inputs)


# revision 32
# speedup vs baseline: 1.1930x; 1.1930x over previous
"""Trainium2 Bass kernel for nn_Attention_3135326126702.

Computation (see reference): ViT-style attention block on x:(2,384,56,56).
  q/k/v/proj are 1x1 conv + eval-mode BN (affine, folded into weights on host).
  8 heads, key_dim=16, d=64, N=3136 positions, softmax(q@k) (no scale) @ v,
  relu, proj conv+BN.

Sharding: 8 cores = (batch b in {0,1}) x (head-pair hp in {0..3}).
Each core computes K/V/Q convs for ITS 2 HEADS only over all N positions
(no duplicated conv work), runs attention for its 2 heads over 4 query
chunks of 784, and projects its 2 heads' contribution to the output --
a PARTIAL [384, N] result. The host sums the 4 partials per batch and
adds the proj bias: proj is linear in heads, so no device collective.

On-chip layout (channels/keys on partitions, no transposes):
  The head block-diagonal lives on the Q side: q_sb[:, h, :] has head h's
  16 q-channels at rows 16*(3h) (h0 rows 0:16, h1 rows 48:64), zeros
  elsewhere; k_all[128, N] holds both heads' K on the matching rows.
  Every scores matmul is a full K=128 contraction (keeps the PE HAM
  clock-gate at 8/8); zero rows make it per-head.
    scoresT[key, q] = matmul(lhsT=k_all[:, key_tile], rhs=q_sb[:, h, qchunk])
  exp is split across TWO engines (the loop is otherwise ScalarE-gated):
  16/25 key tiles use ScalarE exp; 9/25 use a DVE Schraudolph
  approximation -- one tensor_scalar op computing
  int16(s*184.665 + 16251.05), whose bit pattern IS bf16 exp(s) to ~3%
  (softmax ratio structure cancels most of it; measured end-to-end
  rel err ~7e-3 vs the 2e-2 gate).
  xxT_aug[65, q] += matmul(lhsT=VT_aug[key_tile, h, 65] bf16, rhs=exp_scoresT)
    VT_aug col 64 == 1.0 -> row 64 accumulates the softmax denominator.
  normalize: denominator row -> DRAM-bounce partition broadcast,
  fast-approx reciprocal, xx = max(xxT*rec + bv, 0).
  proj partial per query chunk (2 K=64 matmuls/tile), streamed out by DMA
  during the next chunk's attention; only the last chunk is tail.

dtypes: convs/scores/proj fp16, attnV bf16 (exp values reach ~1e7),
partial outputs f32 (summed on host).
"""

import numpy as np

import concourse.bass as bass
import concourse.mybir as mybir
from concourse import bacc
import concourse.tile as tile
from concourse.bass_utils import run_bass_kernel_spmd

F32 = mybir.dt.float32
BF16 = mybir.dt.bfloat16
FP16 = mybir.dt.float16
I16 = mybir.dt.int16
AF = mybir.ActivationFunctionType
ALU = mybir.AluOpType

EPS = 1e-5
B = 2
CIN = 384          # input channels
N = 3136           # 56*56 positions
NQ = 784           # query positions per chunk (N/4)
NH = 8             # heads total
NHL = 2            # heads per core
KD = 16            # key dim per head
NKT = 25           # key tiles of 128 (last has 64)
FCH = ((0, 512), (512, 272))  # 784-wide free dim split at the PSUM bank edge
N_CORES = 8
ROW = (0, 64)      # q/k channel row offset per local head (32-aligned)

# Schraudolph bf16 exp: bits = trunc(s * 2^7/ln2 + (127*2^7 - 5.45 + 0.5))
EXP_A = 184.6649652337873
EXP_B = 16251.05
# exp engine alternates per key tile: odd -> DVE, even -> ScalarE. With the
# lag-2 attnV emission this keeps both exp engines concurrently busy while
# PE streams matmuls back-to-back.
DVE_KT = frozenset(range(1, 24, 2))

_CACHE = {}


def _key_tiles():
    for kt in range(NKT):
        p0 = kt * 128
        yield kt, p0, min(128, N - p0)


def build_nc() -> bass.Bass:
    nc = bacc.Bacc()

    xb = nc.declare_dram_parameter("xb", [CIN, N], FP16, isOutput=False)[:]
    wqT = nc.declare_dram_parameter("wqT", [CIN, 128], FP16, isOutput=False)[:]
    wkT = nc.declare_dram_parameter("wkT", [CIN, 128], FP16, isOutput=False)[:]
    wvT = nc.declare_dram_parameter("wvT", [CIN, 128], FP16, isOutput=False)[:]
    wpT = nc.declare_dram_parameter("wpT", [64, NHL, CIN], FP16, isOutput=False)[:]
    bq = nc.declare_dram_parameter("bq", [128, 1], F32, isOutput=False)[:]
    bv2 = nc.declare_dram_parameter("bv2", [64, NHL], F32, isOutput=False)[:]
    out = nc.declare_dram_parameter("out", [CIN, N], F32, isOutput=True)[:]

    with tile.TileContext(nc) as tc:
        with (
            tc.tile_pool(name="const", bufs=1) as cst,
            tc.tile_pool(name="work", bufs=3) as wk,
            tc.tile_pool(name="ps", bufs=2, space="PSUM") as ps,
            tc.tile_pool(name="dram", bufs=2, space="DRAM") as dr,
        ):
            # ---- constants / weights ----
            # DMA priority: k/q weights and the first x column group come
            # first so the convs (and with them the PE clock ramp) start
            # as early as possible; everything else follows.
            wk_sb = cst.tile([128, 3, 128], FP16)
            nc.sync.dma_start(out=wk_sb, in_=wkT.rearrange("(o p) m -> p o m", p=128))

            # ---- persistent activations ----
            # q_sb[:, h]: head h q-channels at rows ROW[h]..+16, zeros elsewhere
            q_sb = cst.tile([128, NHL, N], FP16)
            k_all = cst.tile([128, N], FP16)       # both heads' K on ROW rows
            vvT_sb = cst.tile([128, NKT, NHL, 65], BF16)  # [key, kt, h, 64 V + ones]
            xx_sb = cst.tile([64, NHL, NQ], FP16)  # per-chunk normalized xx

            nc.vector.memset(k_all.bitcast(F32), 0.0)
            nc.vector.memset(q_sb[:, :, 0:NQ].bitcast(F32), 0.0)
            nc.vector.memset(vvT_sb[:, :, :, 64:65], 1.0)
            for qd in range(1, 4):
                nc.vector.memset(q_sb[:, :, qd * NQ:(qd + 1) * NQ].bitcast(F32), 0.0)

            x_sb = cst.tile([128, 3, N], FP16)
            x_src = xb.rearrange("(o p) n -> p o n", p=128)
            nc.sync.dma_start(out=x_sb[:, 0, 0:NQ], in_=x_src[:, 0, 0:NQ])
            nc.sync.dma_start(out=x_sb[:, 1:3, 0:NQ], in_=x_src[:, 1:3, 0:NQ])

            wq_sb = cst.tile([128, 3, 128], FP16)
            nc.sync.dma_start(out=wq_sb, in_=wqT.rearrange("(o p) m -> p o m", p=128))
            bq_sb = cst.tile([128, 1], F32)
            nc.sync.dma_start(out=bq_sb, in_=bq)
            wv_sb = cst.tile([128, 3, 128], FP16)
            nc.sync.dma_start(out=wv_sb, in_=wvT.rearrange("(o p) m -> p o m", p=128))
            wp_sb = cst.tile([64, NHL, CIN], FP16)
            nc.sync.dma_start(out=wp_sb, in_=wpT)
            bv2_sb = cst.tile([64, NHL], F32)
            nc.sync.dma_start(out=bv2_sb, in_=bv2)
            for qd in range(1, 4):
                nc.sync.dma_start(
                    out=x_sb[:, :, qd * NQ:(qd + 1) * NQ],
                    in_=x_src[:, :, qd * NQ:(qd + 1) * NQ],
                )

            warm = ps.tile([128, 128], F32, tag="sc", bufs=3, name="warm")
            for _ in range(20):
                nc.tensor.matmul(
                    warm, lhsT=wk_sb[:, 0, :], rhs=wk_sb[:, 0, :],
                    start=True, stop=True,
                )

            # ---- emission order == scheduler priority ----

            def k_conv(g):
                # K conv for column group g -> one evacuation copy into k_all
                # (BN bias dropped: per-query-constant shift is softmax-invariant)
                g0 = g * NQ
                for f0, fl in FCH:
                    kp = ps.tile([128, NQ], F32, tag="sc", bufs=3, name="kp")
                    for c in range(3):
                        nc.tensor.matmul(
                            kp[:, :fl],
                            lhsT=wk_sb[:, c, :],
                            rhs=x_sb[:, c, g0 + f0:g0 + f0 + fl],
                            start=(c == 0), stop=(c == 2),
                        )
                    nc.vector.tensor_copy(
                        out=k_all[:, g0 + f0:g0 + f0 + fl], in_=kp[:, :fl],
                    )

            def q_conv(qc):
                # Q conv for one query chunk; scatter the 2 head stripes
                # (same partitions -- engines cannot shift partitions).
                g0 = qc * NQ
                for f0, fl in FCH:
                    qp = ps.tile([128, NQ], F32, tag="sc", bufs=3, name="qp")
                    for c in range(3):
                        nc.tensor.matmul(
                            qp[:, :fl],
                            lhsT=wq_sb[:, c, :],
                            rhs=x_sb[:, c, g0 + f0:g0 + f0 + fl],
                            start=(c == 0), stop=(c == 2),
                        )
                    for h in range(NHL):
                        r = ROW[h]
                        nc.vector.tensor_scalar(
                            out=q_sb[r:r + KD, h, g0 + f0:g0 + f0 + fl],
                            in0=qp[r:r + KD, :fl],
                            scalar1=bq_sb[r:r + KD, 0:1], scalar2=None,
                            op0=ALU.add,
                        )

            def vt_conv(kt, p0, pl):
                # VT conv: out[key_tile, 128] = x_chunk.T @ wv.T. BN bias
                # added after normalization (softmax weights sum to 1).
                vp = ps.tile([128, 128], F32, tag="sc", bufs=3, name="vp")
                for c in range(3):
                    nc.tensor.matmul(
                        vp[:pl, :],
                        lhsT=x_sb[:, c, p0:p0 + pl],
                        rhs=wv_sb[:, c, :],
                        start=(c == 0), stop=(c == 2),
                    )
                nc.vector.tensor_copy(
                    out=vvT_sb[:pl, kt, :, 0:64],
                    in_=vp[:pl, :].rearrange("p (h d) -> p h d", h=NHL),
                )

            # ONE software-pipelined PE stream across ALL units: attnV for
            # slot s is emitted interleaved with the scores of slot s+2
            # (S512(k), A512(k-2), S272(k), A272(k-2)) so that (a) PE never
            # sits out an exp -- it has had ~2 slots of wall clock -- and
            # (b) every 151ns scores-LDWEIGHTS hides under a 512-wide
            # matmul. The lag queue carries across unit boundaries, so unit
            # transitions cost nothing.
            lag = []

            def attn_chunk(xx2, h, es, kt, pl, f0, fl):
                nc.tensor.matmul(
                    xx2[0] if f0 == 0 else xx2[1],
                    lhsT=vvT_sb[:pl, kt, h, :],
                    rhs=es[:pl, f0:f0 + fl],
                    start=(kt == 0), stop=(kt == NKT - 1),
                )

            def unit(h, qc, side_work):
                # the two attnV accumulators are single-buffered one-bank
                # tiles (evacuated to SBUF right after their last attnV by
                # ScalarE), which frees PSUM for sp bufs=3: the chain
                # scores(k) -> exp(k) -> sem -> scores(k+3) then has
                # positive slack and PE streams at full rate.
                q0 = qc * NQ
                xxA = ps.tile([65, 512], F32, tag="xa", bufs=1, name="xxA")
                xxB = ps.tile([65, 272], F32, tag="xb", bufs=1, name="xxB")
                for kt, p0, pl in _key_tiles():
                    if kt in side_work:
                        side_work[kt]()
                    sp = ps.tile([128, NQ], F32, tag="sc", bufs=3, name="sp")
                    nc.tensor.matmul(
                        sp[:pl, 0:512],
                        lhsT=k_all[:, p0:p0 + pl],
                        rhs=q_sb[:, h, q0:q0 + 512],
                        start=True, stop=True,
                    )
                    if len(lag) > 2:
                        attn_chunk(*lag[0], 0, 512)
                    nc.tensor.matmul(
                        sp[:pl, 512:784],
                        lhsT=k_all[:, p0:p0 + pl],
                        rhs=q_sb[:, h, q0 + 512:q0 + NQ],
                        start=True, stop=True,
                    )
                    es = wk.tile([128, NQ], BF16, tag="es", bufs=8, name="es")
                    if kt in DVE_KT:
                        with nc.allow_low_precision(reason="schraudolph exp ~3%"):
                            nc.vector.tensor_scalar(
                                out=es[:pl].bitcast(I16), in0=sp[:pl],
                                scalar1=EXP_A, scalar2=EXP_B,
                                op0=ALU.mult, op1=ALU.add,
                            )
                    else:
                        nc.scalar.activation(out=es[:pl], in_=sp[:pl], func=AF.Exp)
                    if len(lag) > 2:
                        attn_chunk(*lag.pop(0), 512, 272)
                    lag.append(((xxA, xxB), h, es, kt, pl))
                return (xxA, xxB)

            def flush_lag():
                for args in lag:
                    for f0, fl in FCH:
                        attn_chunk(*args, f0, fl)
                lag.clear()

            def evac_parts(xx2):
                # ScalarE copies of the finished accumulators into SBUF;
                # frees the single-buffered PSUM tiles for the next unit
                xxs = wk.tile([65, NQ], F32, tag="xxs", bufs=2, name="xxs")

                def eA():
                    nc.scalar.copy(out=xxs[:, 0:512], in_=xx2[0])

                def eB():
                    nc.scalar.copy(out=xxs[:, 512:784], in_=xx2[1])

                return xxs, eA, eB

            def norm_parts(xxs, h):
                # normalize + relu: xx = max(xxT*rec + bv, 0); denom = row 64.
                # Returned as FOUR side-work pieces spread across the next
                # unit's key tiles, so each ~1us DVE op slots between that
                # unit's exps instead of pushing them past their attnV
                # deadline. The DRAM-bounce broadcast latency is covered by
                # the gap between piece 1 and piece 2.
                st = {}

                def p1():
                    dnd = dr.tile([1, NQ], F32, name="dnd")
                    nc.sync.dma_start(out=dnd, in_=xxs[64:65, :])
                    st["dnb"] = wk.tile([64, NQ], F32, tag="dnb", bufs=2, name="dnb")
                    nc.gpsimd.dma_start(
                        out=st["dnb"], in_=dnd[0, :].partition_broadcast(64)
                    )

                def p2():
                    st["rec"] = wk.tile([64, NQ], F32, tag="rec", bufs=2, name="rec")
                    with nc.allow_low_precision(reason="~18-bit softmax recip"):
                        nc.vector.reciprocal_approx_fast(out=st["rec"], in_=st["dnb"])

                def p3():
                    st["xt"] = wk.tile([64, NQ], FP16, tag="xt", bufs=2, name="xt")
                    nc.vector.tensor_tensor(
                        out=st["xt"], in0=xxs[0:64, :], in1=st["rec"], op=ALU.mult,
                    )

                def p4():
                    nc.vector.tensor_scalar(
                        out=xx_sb[:, h, :], in0=st["xt"],
                        scalar1=bv2_sb[:, h:h + 1],
                        scalar2=0.0, op0=ALU.add, op1=ALU.max,
                    )

                return p1, p2, p3, p4

            def norm_tail(xxs, h):
                dnd = dr.tile([1, NQ], F32, name="dnd")
                nc.sync.dma_start(out=dnd, in_=xxs[64:65, :])
                dnb = wk.tile([64, NQ], F32, tag="dnb", bufs=2, name="dnb")
                nc.sync.dma_start(
                    out=dnb, in_=dnd[0, :].partition_broadcast(64)
                )
                rec = wk.tile([64, NQ], F32, tag="rec", bufs=2, name="rec")
                with nc.allow_low_precision(reason="~18-bit softmax recip"):
                    nc.vector.reciprocal_approx_fast(out=rec, in_=dnb)
                xt = wk.tile([64, NQ], FP16, tag="xt", bufs=2, name="xt")
                nc.vector.tensor_tensor(
                    out=xt, in0=xxs[0:64, :], in1=rec, op=ALU.mult,
                )
                nc.vector.tensor_scalar(
                    out=xx_sb[:, h, :], in0=xt, scalar1=bv2_sb[:, h:h + 1],
                    scalar2=0.0, op0=ALU.add, op1=ALU.max,
                )

            def proj_tile(qc, t):
                # partial proj (this core's 2 heads only) for one output tile
                pp = ps.tile([128, NQ], F32, tag="sc", bufs=3, name="pp")
                for f0, fl in FCH:
                    for h in range(NHL):
                        nc.tensor.matmul(
                            pp[:, f0:f0 + fl],
                            lhsT=wp_sb[:, h, 128 * t:128 * t + 128],
                            rhs=xx_sb[:, h, f0:f0 + fl],
                            start=(h == 0), stop=(h == NHL - 1),
                        )
                ob = wk.tile([128, NQ], F32, tag="ob", bufs=3, name="ob")
                nc.scalar.copy(out=ob, in_=pp)
                nc.sync.dma_start(
                    out=out.rearrange("(o p) n -> p o n", p=128)[
                        :, t, qc * NQ:(qc + 1) * NQ],
                    in_=ob,
                )

            k_conv(0)
            q_conv(0)
            for g in range(1, 4):
                k_conv(g)

            def mknorm(xx2, h, extra=None):
                xxs, eA, eB = evac_parts(xx2)
                p1, p2, p3, p4 = norm_parts(xxs, h)
                sw = {3: eA, 4: eB, 6: p1, 10: p2, 12: p3, 14: p4}
                if extra:
                    sw.update(extra)
                return sw

            def mkproj(qc):
                return {16: lambda: proj_tile(qc, 0),
                        19: lambda: proj_tile(qc, 1),
                        22: lambda: proj_tile(qc, 2)}

            # first unit races the VT conv tile-by-tile; each unit's
            # normalize runs as split side work inside the NEXT unit
            xp = unit(0, 0, {kt: (lambda kt=kt, p0=p0, pl=pl: vt_conv(kt, p0, pl))
                             for kt, p0, pl in _key_tiles()})
            xp = unit(1, 0, mknorm(xp, 0, {18: lambda: q_conv(1)}))
            xp = unit(0, 1, mknorm(xp, 1, mkproj(0)))
            xp = unit(1, 1, mknorm(xp, 0, {18: lambda: q_conv(2)}))
            xp = unit(0, 2, mknorm(xp, 1, mkproj(1)))
            xp = unit(1, 2, mknorm(xp, 0, {18: lambda: q_conv(3)}))
            xp = unit(0, 3, mknorm(xp, 1, mkproj(2)))
            xp = unit(1, 3, mknorm(xp, 0))
            flush_lag()
            xxs_t, eA_t, eB_t = evac_parts(xp)
            eA_t()
            eB_t()
            norm_tail(xxs_t, 1)
            for t in range(3):
                proj_tile(3, t)

    nc.compile()
    return nc


def _fold_bn(w, g, b, m, v):
    s = (g / np.sqrt(v + EPS)).astype(np.float32)
    return (s[:, None] * w).astype(np.float32), (b - m * s).astype(np.float32)


def make_in_maps(inputs):
    """Host-side prep: fold BN, slice per head-pair, pad q/k rows."""
    wq, bq_ = _fold_bn(inputs["wq"], inputs["qg"], inputs["qb"], inputs["qm"], inputs["qv"])
    wkm, _ = _fold_bn(inputs["wk"], inputs["kg"], inputs["kb"], inputs["km"], inputs["kvv"])
    wv, bv_ = _fold_bn(inputs["wv"], inputs["vg"], inputs["vb"], inputs["vm"], inputs["vvv"])
    wp, _ = _fold_bn(inputs["wp"], inputs["pg"], inputs["pb"], inputs["pm"], inputs["pvv"])

    x = np.asarray(inputs["x"], np.float32).reshape(B, CIN, N)
    xb16 = [np.ascontiguousarray(x[b]).astype(np.float16) for b in range(B)]

    in_maps = []
    for core in range(N_CORES):
        b, hp = divmod(core, 4)
        h0 = 2 * hp
        # q/k weights for local heads at rows ROW[h]..+16 of a 128-row block
        wq2 = np.zeros((128, CIN), np.float32)
        wk2 = np.zeros((128, CIN), np.float32)
        bq2 = np.zeros((128, 1), np.float32)
        for hl in range(NHL):
            h = h0 + hl
            r = ROW[hl]
            wq2[r:r + KD] = wq[KD * h:KD * h + KD]
            wk2[r:r + KD] = wkm[KD * h:KD * h + KD]
            bq2[r:r + KD, 0] = bq_[KD * h:KD * h + KD]
        wv2 = wv[64 * h0:64 * h0 + 128]              # [128, CIN]
        wp2 = wp[:, 64 * h0:64 * h0 + 128]           # [CIN, 128]
        bv22 = bv_[64 * h0:64 * h0 + 128]
        in_maps.append({
            "xb": xb16[b],
            "wqT": np.ascontiguousarray(wq2.T).astype(np.float16),
            "wkT": np.ascontiguousarray(wk2.T).astype(np.float16),
            "wvT": np.ascontiguousarray(wv2.T).astype(np.float16),
            "wpT": np.ascontiguousarray(
                wp2.T.reshape(NHL, 64, CIN).transpose(1, 0, 2)).astype(np.float16),
            "bq": bq2,
            "bv2": np.ascontiguousarray(bv22.reshape(NHL, 64).T),  # [64, 2]
        })
    return in_maps


def assemble(results, inputs):
    _, bp_ = _fold_bn(inputs["wp"], inputs["pg"], inputs["pb"], inputs["pm"], inputs["pvv"])
    out = np.empty((B, CIN, N), np.float32)
    for b in range(B):
        acc = results[4 * b]["out"].astype(np.float32)
        for hp in range(1, 4):
            acc = acc + results[4 * b + hp]["out"]
        out[b] = acc + bp_[:, None]
    return out.reshape(B, CIN, 56, 56)


def kernel(**inputs) -> np.ndarray:
    if "nc" not in _CACHE:
        _CACHE["nc"] = build_nc()
    nc = _CACHE["nc"]
    in_maps = make_in_maps(inputs)
    res = run_bass_kernel_spmd(nc, in_maps, core_ids=list(range(N_CORES)))
    return assemble(res.results, inputs)
